# revision 1
# baseline (speedup 1.0000x reference)
"""CRF negative log-likelihood on 8 Trainium2 NeuronCores.

Strategy
--------
The dominant cost is the forward algorithm (log-partition): a length-T
recurrence of "log-matmuls"  alpha_t = em_t + LSE_i(alpha_{t-1} + trans).
In exp-domain this is  u_t = exp(em_t - c) * (expT^T @ u_{t-1}), i.e. a
128x128 matmul + elementwise multiply per step.

A naive implementation is latency-bound (1024 serial PE->DVE hops). But
transitions are in [-0.1, 0.1], so the positive matrix exp(trans) is a
strong Hilbert-metric contraction (factor ~tanh(0.1) ~ 0.1 per step):
the recurrence forgets its initial condition in ~8 steps. We therefore
split T into C chunks per core, warm each chunk up from a ones-vector
W steps early, and run all chunks in lockstep as columns of ONE state
block [128 states x C*32 cols]. Each "virtual step" is then a single
128x128x512 matmul + one [128,512] multiply - throughput-bound.

Per-chunk log-gains are recovered from boundary column-sums (computed
with a ones/exp(end) matmul) and telescoped into log_Z on the host in
f64. The gold-path score (pure gathers, ~0.006% of FLOPs) and the final
mean are computed on the host.

Sharding: data-parallel over batch B: core i owns b in [32*i, 32*i+32).
"""

import numpy as np
from contextlib import ExitStack

import concourse.bass as bass
import concourse.tile as tile
from concourse import bacc, mybir
from concourse.bass_utils import run_bass_kernel_spmd

# Problem shape (hardcoded per harness contract).
B, T, K = 256, 1024, 128
N_CORES = 8
BC = B // N_CORES          # 32 batch rows per core
C = 16                     # time chunks per core
TC = T // C                # 64 steps per chunk
W = 8                      # warmup steps per chunk
NV = TC + W - 1            # 71 matmul virtual-steps
COLS = C * BC              # 512 state columns per core
CSHIFT = float(np.log(128.0) + 0.5)  # per-step rescale (exactness-neutral)

F32 = mybir.dt.float32

_NC_CACHE = None


def _build_program(repeat=1):
    """Build the per-core SPMD Bass program (identical on all cores).

    repeat > 1 wraps the whole computation in an on-device loop — used
    only by the test harness for differential HW timing.
    """
    nc = bacc.Bacc("TRN2", target_bir_lowering=False, debug=False,
                   num_devices=N_CORES)

    emx = nc.dram_tensor("emx", [K, NV * COLS], F32, kind="ExternalInput").ap()
    trans = nc.dram_tensor("trans", [K, K], F32, kind="ExternalInput").ap()
    stend = nc.dram_tensor("stend", [K, 2], F32, kind="ExternalInput").ap()
    sums = nc.dram_tensor("sums", [2, 2 * COLS], F32,
                          kind="ExternalOutput").ap()

    with tile.TileContext(nc) as tc, ExitStack() as ctx:
        const_pool = ctx.enter_context(tc.tile_pool(name="const", bufs=1))
        raw_pool = ctx.enter_context(tc.tile_pool(name="raw", bufs=4))
        state_pool = ctx.enter_context(tc.tile_pool(name="state", bufs=2))
        psum_pool = ctx.enter_context(
            tc.tile_pool(name="psum", bufs=2, space="PSUM"))
        bsum_pool = ctx.enter_context(
            tc.tile_pool(name="bsum", bufs=2, space="PSUM"))

        # Bias tiles (activation's float-bias path needs a const-AP db;
        # simpler to pass explicit per-partition bias APs).
        bias0 = const_pool.tile([K, 1], F32)
        nc.vector.memset(bias0[:], 0.0)
        biasc = const_pool.tile([K, 1], F32)
        nc.vector.memset(biasc[:], -CSHIFT)

        # Constants: expT (matmul lhsT), [ones | exp(end)] lhsT, exp(start).
        trans_raw = const_pool.tile([K, K], F32)
        nc.sync.dma_start(trans_raw[:], trans[:])
        expT = const_pool.tile([K, K], F32)
        nc.scalar.activation(expT[:], trans_raw[:],
                             mybir.ActivationFunctionType.Exp, bias=bias0[:])

        stend_raw = const_pool.tile([K, 2], F32)
        nc.sync.dma_start(stend_raw[:], stend[:])
        onesend = const_pool.tile([K, 2], F32)
        nc.vector.memset(onesend[:, 0:1], 1.0)
        nc.scalar.activation(onesend[:, 1:2], stend_raw[:, 1:2],
                             mybir.ActivationFunctionType.Exp, bias=bias0[:])
        startexp = const_pool.tile([K, 1], F32)
        nc.scalar.activation(startexp[:], stend_raw[:, 0:1],
                             mybir.ActivationFunctionType.Exp, bias=bias0[:])

        # row0 = [entry sums | final 1^T sums]; row1 = [unused | final end^T]
        out_sb = const_pool.tile([2, 2 * COLS], F32)

        loop_cm = tc.For_i(0, repeat, 1) if repeat > 1 else None
        if loop_cm is not None:
            ctx.enter_context(loop_cm)

        v = state_pool.tile([K, COLS], F32)
        nc.vector.memset(v[:], 1.0)

        for s in range(1, NV + 1):
            e_t = raw_pool.tile([K, COLS], F32)
            nc.sync.dma_start(e_t[:], emx[:, (s - 1) * COLS:s * COLS])
            # exp in place: keeps ACT's semaphore-wait count within the
            # 2-wait hardware limit (no extra tile-slot WAR dependency).
            nc.scalar.activation(e_t[:], e_t[:],
                                 mybir.ActivationFunctionType.Exp,
                                 bias=biasc[:])

            ps = psum_pool.tile([K, COLS], F32)
            nc.tensor.matmul(ps[:], expT[:], v[:], start=True, stop=True)

            v = state_pool.tile([K, COLS], F32)
            nc.vector.tensor_mul(v[:], ps[:], e_t[:])

            if s == W:
                # chunk 0 exact init at t=0: u0 = exp(start) * exp(em0 - c)
                nc.vector.tensor_scalar_mul(v[:, 0:BC], e_t[:, 0:BC],
                                            startexp[:])
            if s == W - 1:
                # entry boundary sums: 1^T v  (state time = c*TC - 1)
                bp = bsum_pool.tile([2, COLS], F32)
                nc.tensor.matmul(bp[:], onesend[:], v[:], start=True,
                                 stop=True)
                nc.vector.tensor_copy(out_sb[0:1, 0:COLS], bp[0:1, :])

        # final boundary sums: [1^T v ; exp(end)^T v]
        bp = bsum_pool.tile([2, COLS], F32)
        nc.tensor.matmul(bp[:], onesend[:], v[:], start=True, stop=True)
        nc.vector.tensor_copy(out_sb[0:2, COLS:2 * COLS], bp[0:2, :])

        nc.sync.dma_start(sums[:], out_sb[:])

    nc.compile()
    return nc


def _host_prep(emissions):
    """Per-core replicated emission layout emx[k, (s-1)*COLS + c*BC + b]
    = em[core*BC + b, clip(c*TC - W + s, 0, T-1), k]."""
    s_idx = np.arange(1, NV + 1)
    c_idx = np.arange(C)
    tau = np.clip(c_idx[None, :] * TC - W + s_idx[:, None], 0, T - 1)  # [NV, C]
    in_maps = []
    for core in range(N_CORES):
        emc = emissions[core * BC:(core + 1) * BC]          # [BC, T, K]
        emT = np.ascontiguousarray(emc.transpose(2, 1, 0))  # [K, T, BC]
        emx = emT[:, tau, :].reshape(K, NV * COLS)
        in_maps.append({"emx": np.ascontiguousarray(emx)})
    return in_maps


def _gold_score(em, tags, mask, trans, start, end):
    em = em.astype(np.float64)
    mask = mask.astype(np.float64)
    tg = tags.astype(np.int64)
    score = start.astype(np.float64)[tg[:, 0]]
    emit = np.take_along_axis(em, tg[:, :, None], axis=2)[:, :, 0]
    score = score + (emit * mask).sum(axis=1)
    score = score + (trans.astype(np.float64)[tg[:, :-1], tg[:, 1:]]
                     * mask[:, 1:]).sum(axis=1)
    seq_ends = mask.astype(np.int64).sum(axis=1) - 1
    last = tg[np.arange(tg.shape[0]), seq_ends]
    score = score + end.astype(np.float64)[last]
    return score


def _host_logz_fallback(em, trans, start, end):
    """Exact f64 forward algorithm (only used if mask is not all-ones)."""
    em = em.astype(np.float64)
    la = start.astype(np.float64) + em[:, 0, :]
    tr = trans.astype(np.float64)
    for t in range(1, em.shape[1]):
        sc = tr[None] + la[:, :, None] + em[:, t, None, :]
        m = sc.max(axis=1, keepdims=True)
        la = np.squeeze(m, 1) + np.log(np.exp(sc - m).sum(axis=1))
    x = la + end[None].astype(np.float64)
    m = x.max(axis=1, keepdims=True)
    return np.squeeze(m, 1) + np.log(np.exp(x - m).sum(axis=1))


def kernel(emissions, tags, mask, transitions, start_transitions,
           end_transitions):
    global _NC_CACHE
    emissions = np.ascontiguousarray(np.asarray(emissions, dtype=np.float32))
    tags = np.asarray(tags)
    mask = np.asarray(mask)
    transitions = np.asarray(transitions, dtype=np.float32)
    start_transitions = np.asarray(start_transitions, dtype=np.float32)
    end_transitions = np.asarray(end_transitions, dtype=np.float32)

    score = _gold_score(emissions, tags, mask, transitions,
                        start_transitions, end_transitions)

    if not np.all(mask == 1):
        logz = _host_logz_fallback(emissions, transitions,
                                   start_transitions, end_transitions)
        return np.float32(-(score - logz).mean())

    if _NC_CACHE is None:
        _NC_CACHE = _build_program()
    nc = _NC_CACHE

    in_maps = _host_prep(emissions)
    trans_in = np.ascontiguousarray(transitions)
    stend_in = np.ascontiguousarray(
        np.stack([start_transitions, end_transitions], axis=1))
    for m in in_maps:
        m["trans"] = trans_in
        m["stend"] = stend_in

    results = run_bass_kernel_spmd(nc, in_maps, list(range(N_CORES))).results

    # Host assembly in f64: telescoped per-chunk log-gains.
    logz = np.zeros(B)
    for core in range(N_CORES):
        r = np.asarray(results[core]["sums"], dtype=np.float64)
        entry = r[0, :COLS].reshape(C, BC)
        end0 = r[0, COLS:].reshape(C, BC)
        end1 = r[1, COLS:].reshape(C, BC)
        acc = np.log(end0[0]).copy()                      # chunk 0: exact scale
        for c in range(1, C - 1):
            acc += np.log(end0[c]) - np.log(entry[c])
        acc += np.log(end1[C - 1]) - np.log(entry[C - 1])  # last: exp(end)^T
        logz[core * BC:(core + 1) * BC] = acc + T * CSHIFT

    return np.float32(-(score - logz).mean())

